# revision 24
# baseline (speedup 1.0000x reference)
"""BatchAllTripletLoss on 8 Trainium2 NeuronCores via Bass/Tile.

Math: for anchors i, positives j (same label, j!=i), negatives k (diff label):
  total        = sum_{i,j,k} relu(d_ij - d_ik + margin)
  num_non_easy = #{(i,j,k): d_ik < d_ij + margin}
  loss         = total / num_non_easy ; frac = num_non_easy / num_valid

Sharding: samples are sorted by class; core r owns 80 consecutive anchors of
the sorted order. Each core additionally gets its OWN cyclic rotation of the
sorted sample order such that the classes its anchors belong to occupy
columns [0, W) with W <= 256 ("window"). All positives j of every local
anchor then live in the first NCT_W=2 column tiles, so the per-anchor
comparison matrix M[j, k] = (v'_k < t'_j) shrinks from [640, 640] to
[256, 640]. Loss sums/counts are permutation invariant, so each core can use
its own sample order.

Per anchor a:
  - masked row v'_k = d_ak + BIG*(same label), bf16, staged to DRAM and
    DMA-broadcast to [128, 8*640] for 8 anchors at a time.
  - masked thresholds t'_j = (d_aj + margin) * positive_mask over the window
    rows only, hi/lo split into bf16 lhsT tiles.
  - M built in one bf16 op per window tile: DVE tensor_scalar is_lt for 5 of
    8 anchors; ACT Sign (corrected on host) for 3 of 8.
  - PE reduces M with lhsT = [t'_hi | t'_lo | 1 | 0] (bf16), psum rows 4m..
    4m+3 for anchor m of a 32-anchor psum group (32 anchors share one bank
    pair; drains happen 3x per core instead of 27x).
  - drain: ACT free-sums psum rows; DVE fused (q * dist) reduce.
  total = sum(t'*M) - sum(d*q);  count = sum(q).  Host combines in f64.
num_valid is pure label counting (host, exact).
"""

import numpy as np
import ml_dtypes

N = 640
D = 128
NCORES = 8
NLOC = N // NCORES            # 80 anchors per core
GRPA = 24                     # anchors per psum group (3 quadrants x 8)
NGRP = (NLOC + GRPA - 1) // GRPA   # 4 groups: 24, 24, 24, 8
VBB = 8                       # anchors per v'-broadcast DMA
MARGIN = 1.9
BIG = 1.0e9


def _is_sign_anchor(a: int) -> bool:
    return a % 8 in (0, 3)


def _is_pool_anchor(a: int) -> bool:
    return a % 8 in (6, 7)


_CACHE = {}


def _build_program(nct_w: int):
    import concourse.bass as bass
    import concourse.bacc as bacc
    import concourse.mybir as mybir
    import concourse.tile as tile
    from concourse.masks import make_identity

    f32 = mybir.dt.float32
    bf16 = mybir.dt.bfloat16
    Alu = mybir.AluOpType
    Act = mybir.ActivationFunctionType

    W = nct_w * 128               # window width (positive-candidate columns)

    nc = bacc.Bacc("TRN2", target_bir_lowering=False, debug=False,
                   num_devices=NCORES)

    efT = nc.declare_dram_parameter("efT", [D, N], f32, isOutput=False)
    elocT = nc.declare_dram_parameter("elocT", [D, NLOC], f32, isOutput=False)
    labrow = nc.declare_dram_parameter("labrow", [1, N], f32, isOutput=False)
    labT = nc.declare_dram_parameter("labT", [128, nct_w], f32, isOutput=False)
    llocrow = nc.declare_dram_parameter("llocrow", [1, NLOC], f32, isOutput=False)
    llocT = nc.declare_dram_parameter("llocT", [NLOC, 1], f32, isOutput=False)
    eye = nc.declare_dram_parameter("eye", [128, nct_w * NLOC], f32,
                                    isOutput=False)
    onesb = nc.declare_dram_parameter("onesb", [128, NLOC], bf16,
                                      isOutput=False)
    # out: [128, 2*NGRP(wsums) + 2*NGRP(p2) + 1 (dist row sums)]
    out_d = nc.declare_dram_parameter("out", [128, 4 * NGRP + 1], f32,
                                      isOutput=True)
    # out2: [1, NLOC] Tsum_a = sum_j t'_aj (for ACT-sign anchors' corrections)
    out2_d = nc.declare_dram_parameter("out2", [1, NLOC], f32, isOutput=True)

    from contextlib import ExitStack
    with tile.TileContext(nc) as tc:
        with (
            tc.tile_pool(name="singles", bufs=1) as sg,
            tc.tile_pool(name="vbp", bufs=3) as vbp,
            tc.tile_pool(name="mtp", bufs=10) as mtp,
            tc.tile_pool(name="dpp", bufs=2) as dpp,
            tc.tile_pool(name="drs", bufs=3) as drs,
            tc.tile_pool(name="dram", bufs=1, space="DRAM") as dram,
        ):
            pro_stack = ExitStack()
            ps_mm = pro_stack.enter_context(
                tc.tile_pool(name="ps_mm", bufs=1, space="PSUM"))
            ps_tr = pro_stack.enter_context(
                tc.tile_pool(name="ps_tr", bufs=1, space="PSUM"))
            # ---- load inputs ----
            EF = sg.tile([D, N], f32)
            nc.sync.dma_start(out=EF[:], in_=efT[:])
            EL = sg.tile([D, NLOC], f32)
            nc.sync.dma_start(out=EL[:], in_=elocT[:])
            LR = sg.tile([1, N], f32)
            nc.gpsimd.dma_start(out=LR[:], in_=labrow[:])
            LT = sg.tile([128, nct_w], f32)
            nc.gpsimd.dma_start(out=LT[:], in_=labT[:])
            LLR = sg.tile([1, NLOC], f32)
            nc.gpsimd.dma_start(out=LLR[:], in_=llocrow[:])
            LLT = sg.tile([NLOC, 1], f32)
            nc.gpsimd.dma_start(out=LLT[:], in_=llocT[:])
            EYE = sg.tile([128, nct_w * NLOC], f32)
            nc.sync.dma_start(out=EYE[:], in_=eye[:])
            # pre-allocate + zero the quadrant lhsT tiles early (no data dep)
            lhs32 = []
            row_d = 32 * NLOC
            for c in range(nct_w):
                L32 = sg.tile([128, 32, NLOC], bf16, tag=f"l32_{c}",
                              name=f"l32_{c}")
                nc.gpsimd.memset(L32[:], 0.0)
                for s in range(8):
                    dst_o = bass.AP(tensor=L32[:].tensor,
                                    offset=L32[:].offset + (16 + s) * NLOC + s,
                                    ap=[[row_d, 128], [8, NLOC // 8]])
                    nc.gpsimd.memset(dst_o, 1.0)
                lhs32.append(L32)

            ident = sg.tile([128, 128], f32)
            make_identity(nc, ident[:])
            ones = sg.tile([128, 1], f32)
            nc.vector.memset(ones[:], 1.0)

            # ---- pairwise distance rows for local anchors ----
            Esq = sg.tile([D, N], f32)
            nc.vector.tensor_mul(Esq[:], EF[:], EF[:])
            ELsq = sg.tile([D, NLOC], f32)
            nc.vector.tensor_mul(ELsq[:], EL[:], EL[:])

            sqf_ps = ps_mm.tile([1, N], f32, tag="pro", name="sqf")
            nc.tensor.matmul(sqf_ps[:, 0:512], ones[:], Esq[:, 0:512])
            nc.tensor.matmul(sqf_ps[:, 512:N], ones[:], Esq[:, 512:N])
            SQF = sg.tile([1, N], f32)
            nc.vector.tensor_copy(SQF[:], sqf_ps[:])

            sql_ps = ps_mm.tile([NLOC, 1], f32, tag="pro", name="sql")
            nc.tensor.matmul(sql_ps[:], ELsq[:], ones[:])
            SQL = sg.tile([NLOC, 1], f32)
            nc.vector.tensor_copy(SQL[:], sql_ps[:])

            dot_ps = ps_mm.tile([NLOC, N], f32, tag="pro", name="dot")
            nc.tensor.matmul(dot_ps[:, 0:512], EL[:], EF[:, 0:512])
            nc.tensor.matmul(dot_ps[:, 512:N], EL[:], EF[:, 512:N])

            A = sg.tile([NLOC, N], f32)
            nc.vector.tensor_scalar(out=A[:], in0=dot_ps[:], scalar1=-2.0,
                                    scalar2=SQL[:], op0=Alu.mult, op1=Alu.add)
            sqf_d = dram.tile([1, N], f32)
            nc.sync.dma_start(out=sqf_d[:], in_=SQF[:])
            SQB = sg.tile([128, N], f32)
            nc.sync.dma_start(out=SQB[0:NLOC, :],
                              in_=sqf_d[:].to_broadcast([NLOC, N]))
            PRE = sg.tile([NLOC, N], f32)
            nc.vector.tensor_add(PRE[:], A[:], SQB[0:NLOC, :])
            nc.vector.tensor_scalar(out=PRE[:], in0=PRE[:], scalar1=0.0,
                                    scalar2=None, op0=Alu.max)
            DIST = sg.tile([NLOC, N], f32)
            nc.scalar.activation(out=DIST[:], in_=PRE[:], func=Act.Sqrt)

            # masked v' row, bf16
            LBC = sg.tile([128, N], f32)
            nc.sync.dma_start(out=LBC[0:NLOC, :],
                              in_=labrow[:].to_broadcast([NLOC, N]))
            EQB = sg.tile([NLOC, N], f32)
            nc.vector.tensor_scalar(out=EQB[:], in0=LBC[0:NLOC, :], scalar1=LLT[:],
                                    scalar2=BIG, op0=Alu.is_equal, op1=Alu.mult)
            VM = sg.tile([NLOC, N], f32)
            nc.vector.tensor_add(VM[:], DIST[:], EQB[:])
            VMB = sg.tile([NLOC, N], bf16)
            nc.vector.tensor_copy(VMB[:], VM[:])
            vmd = dram.tile([NLOC, N], bf16)
            nc.sync.dma_start(out=vmd[:], in_=VMB[:])

            # positive mask transposed: (lab_j == lab_a) - eye  (window only)
            LLB = sg.tile([128, NLOC], f32)
            nc.sync.dma_start(out=LLB[:],
                              in_=llocrow[:].to_broadcast([128, NLOC]))
            posT = []
            for c in range(nct_w):
                p = sg.tile([128, NLOC], f32, tag=f"posT{c}", name=f"posT{c}")
                nc.vector.tensor_scalar(out=p[:], in0=LLB[:], scalar1=LT[:, c:c + 1],
                                        scalar2=None, op0=Alu.is_equal)
                nc.vector.tensor_sub(p[:], p[:], EYE[:, c * NLOC:(c + 1) * NLOC])
                posT.append(p)

            # thresholds: tp[c][p, a] = (dist[a, c*128+p] + margin) * posT
            # plus bf16 hi/lo split packed into lhsT tiles [128, NLOC, 4],
            # then scattered via DRAM into zero-padded [128, NLOC, 32] tiles
            # so 8 anchors can share one PE output quadrant (anchor a's 4
            # cols live at 4*(a%8); the rest are zeros).
            tp = []
            for c in range(nct_w):
                tr_ps = ps_tr.tile([128, NLOC], f32, tag="tr")
                nc.tensor.transpose(tr_ps[:], DIST[:, c * 128:(c + 1) * 128],
                                    ident[0:NLOC, 0:NLOC])
                t = sg.tile([128, NLOC], f32, tag=f"tp{c}", name=f"tp{c}")
                nc.vector.tensor_scalar_add(out=t[:], in0=tr_ps[:], scalar1=MARGIN)
                nc.vector.tensor_mul(t[:], t[:], posT[c][:])
                tp.append(t)

                TH = sg.tile([128, NLOC], bf16, tag="thb", name="thb")
                nc.vector.tensor_copy(TH[:], t[:])                 # t_hi (bf16)
                thf = sg.tile([128, NLOC], f32, tag="thf", name="thf")
                nc.vector.tensor_copy(thf[:], TH[:])               # back to f32
                nc.vector.tensor_sub(thf[:], t[:], thf[:])         # t_lo
                TL = sg.tile([128, NLOC], bf16, tag="tlb", name="tlb")
                nc.vector.tensor_copy(TL[:], thf[:])
                # fill zero-padded quadrant lhsT (layout [128, col, a]):
                # anchor slot s=a%8 has (hi, lo) at cols (2s, 2s+1); ones
                # pre-filled at col 16+s. matmul reads col-strided.
                L32 = lhs32[c]
                row_s = NLOC
                for s in range(8):
                    nhere = NLOC // 8
                    src_h = bass.AP(tensor=TH[:].tensor, offset=TH[:].offset + s,
                                    ap=[[row_s, 128], [8, nhere]])
                    dst_h = bass.AP(tensor=L32[:].tensor,
                                    offset=L32[:].offset + 2 * s * NLOC + s,
                                    ap=[[row_d, 128], [8, nhere]])
                    nc.vector.tensor_copy(dst_h, src_h)
                    src_l = bass.AP(tensor=TL[:].tensor, offset=TL[:].offset + s,
                                    ap=[[row_s, 128], [8, nhere]])
                    dst_l = bass.AP(tensor=L32[:].tensor,
                                    offset=L32[:].offset + (2 * s + 1) * NLOC + s,
                                    ap=[[row_d, 128], [8, nhere]])
                    nc.gpsimd.tensor_copy(dst_l, src_l)

            # dist row sums (for sign-anchor corrections) -> OUTS directly
            OUTS = sg.tile([128, 4 * NGRP + 1], f32)
            DSC = sg.tile([NLOC, N], f32)
            nc.scalar.activation(out=DSC[:], in_=DIST[:], func=Act.Identity,
                                 bias=0.0, scale=1.0,
                                 accum_out=OUTS[0:NLOC, 4 * NGRP:4 * NGRP + 1])

            # Tsum_a = sum_j t'_aj : ones^T @ tp[c], accumulated over c
            ts_ps = ps_tr.tile([1, NLOC], f32, tag="tr", name="ts_ps")
            for c in range(nct_w):
                nc.tensor.matmul(ts_ps[:], ones[:], tp[c][:],
                                 start=(c == 0), stop=(c == nct_w - 1))
            TSROW = sg.tile([1, NLOC], f32)
            nc.vector.tensor_copy(TSROW[:], ts_ps[:])
            nc.sync.dma_start(out=out2_d[:], in_=TSROW[:])

            pro_stack.close()
            wq_stack = ExitStack()
            ps_wq1 = wq_stack.enter_context(
                tc.tile_pool(name="ps_wq1", bufs=2, space="PSUM"))
            ps_wq2 = wq_stack.enter_context(
                tc.tile_pool(name="ps_wq2", bufs=2, space="PSUM"))

            # ---- main loop ----
            vb_cache = {}
            for g in range(NGRP):
                na = min(GRPA, NLOC - GRPA * g)
                nqd = (na + 7) // 8
                wq1 = ps_wq1.tile([128, 512], f32, tag="wq1", name="wq1")
                wq2 = ps_wq2.tile([128, 128], f32, tag="wq2", name="wq2")
                dp = dpp.tile([128, N], f32, tag="dp", name="dp")
                # dist rows of quadrant's anchors -> dp partitions 32*qd+16+s
                for qd in range(nqd):
                    bn = min(8, na - 8 * qd)
                    a0 = GRPA * g + 8 * qd
                    nc.sync.dma_start(
                        out=dp[32 * qd + 16:32 * qd + 16 + bn, :],
                        in_=DIST[a0:a0 + bn, :])
                for m in range(na):
                    a = GRPA * g + m
                    qd, s8 = m // 8, m % 8
                    bn = min(8, na - 8 * qd)
                    if a % VBB == 0:
                        vb2 = vbp.tile([128, VBB, N], bf16, tag="vb", name="vb")
                        sl = vmd[a:a + VBB, :]
                        bsrc = bass.AP(tensor=sl.tensor, offset=sl.offset,
                                       ap=[[0, 128]] + [list(q) for q in sl.ap])
                        nc.sync.dma_start(out=vb2[:], in_=bsrc)
                        vb_cache[0] = vb2
                    vb = vb_cache[0][:, a % VBB, :]
                    on_act = _is_sign_anchor(a)
                    on_pool = _is_pool_anchor(a)
                    first = (s8 == 0)
                    last = (s8 == bn - 1)
                    for c in range(nct_w):
                        mt = mtp.tile([128, N], bf16, tag="mt", name="mt")
                        if on_act:
                            nc.scalar.activation(out=mt[:], in_=vb[:],
                                                 func=Act.Sign,
                                                 bias=tp[c][:, a:a + 1],
                                                 scale=-1.0)
                        elif on_pool:
                            nc.gpsimd.tensor_scalar(out=mt[:], in0=vb[:],
                                                    scalar1=tp[c][:, a:a + 1],
                                                    scalar2=None, op0=Alu.is_lt)
                        else:
                            nc.vector.tensor_scalar(out=mt[:], in0=vb[:],
                                                    scalar1=tp[c][:, a:a + 1],
                                                    scalar2=None, op0=Alu.is_lt)
                        st = first and (c == 0)
                        sp = last and (c == nct_w - 1)
                        nc.tensor.matmul(wq1[32 * qd:32 * qd + 32, :],
                                         lhs32[c][:, :, a], mt[:, 0:512],
                                         start=st, stop=sp)
                        nc.tensor.matmul(wq2[32 * qd:32 * qd + 32, :],
                                         lhs32[c][:, :, a], mt[:, 512:N],
                                         start=st, stop=sp)
                # drain group: ACT free-sums all psum rows; DVE fused q*dist
                sa1 = drs.tile([128, 512], f32, tag="sa1", name="sa1")
                sa2 = drs.tile([128, 128], f32, tag="sa2", name="sa2")
                sb1 = drs.tile([128, 512], f32, tag="sb1", name="sb1")
                sb2 = drs.tile([128, 128], f32, tag="sb2", name="sb2")
                nc.scalar.activation(out=sa1[:], in_=wq1[:], func=Act.Identity,
                                     bias=0.0, scale=1.0,
                                     accum_out=OUTS[:, 2 * g:2 * g + 1])
                nc.scalar.activation(out=sa2[:], in_=wq2[:], func=Act.Identity,
                                     bias=0.0, scale=1.0,
                                     accum_out=OUTS[:, 2 * g + 1:2 * g + 2])
                nc.vector.scalar_tensor_tensor(out=sb1[:], in0=wq1[:],
                                               scalar=1.0, in1=dp[:, 0:512],
                                               op0=Alu.mult, op1=Alu.mult,
                                               accum_out=OUTS[:, 2 * NGRP + 2 * g:
                                                              2 * NGRP + 2 * g + 1])
                nc.vector.scalar_tensor_tensor(out=sb2[:], in0=wq2[:],
                                               scalar=1.0, in1=dp[:, 512:N],
                                               op0=Alu.mult, op1=Alu.mult,
                                               accum_out=OUTS[:, 2 * NGRP + 2 * g + 1:
                                                              2 * NGRP + 2 * g + 2])

            nc.gpsimd.dma_start(out=out_d[:], in_=OUTS[:])
            wq_stack.close()

    nc.compile()
    return nc


def _get_program(nct_w: int):
    key = ("nc", nct_w)
    if key not in _CACHE:
        _CACHE[key] = _build_program(nct_w)
    return _CACHE[key]


def _plan_layout(lab: np.ndarray):
    """Class-sort the samples; per core build a rotated order that puts the
    window (classes of its anchors) at columns [0, W)."""
    order = np.argsort(lab, kind="stable")
    slab = lab[order]
    perms = []
    aoffs = []
    maxw = 0
    for r in range(NCORES):
        lo = slab[NLOC * r]
        hi = slab[NLOC * r + NLOC - 1]
        w0 = int(np.searchsorted(slab, lo, "left"))
        w1 = int(np.searchsorted(slab, hi, "right"))
        maxw = max(maxw, w1 - w0)
        perm = np.concatenate([order[w0:], order[:w0]])
        perms.append(perm)
        aoffs.append(NLOC * r - w0)
    nct_w = max(1, (maxw + 127) // 128)
    return perms, aoffs, nct_w


def _make_inputs(embeddings: np.ndarray, labels: np.ndarray):
    e = np.ascontiguousarray(embeddings.reshape(N, D).astype(np.float32))
    lab = labels.reshape(N).astype(np.float32)
    perms, aoffs, nct_w = _plan_layout(lab)

    in_maps = []
    for r in range(NCORES):
        perm = perms[r]
        aoff = aoffs[r]
        er = e[perm]                                     # [N, D] core order
        efT = np.ascontiguousarray(er.T)                 # [D, N]
        labr = lab[perm]
        eye = np.zeros((128, nct_w * NLOC), np.float32)
        for a in range(NLOC):
            j = aoff + a
            eye[j % 128, (j // 128) * NLOC + a] = 1.0
        labT = np.zeros((128, nct_w), np.float32)
        labT[:, :] = labr[:nct_w * 128].reshape(nct_w, 128).T
        in_maps.append({
            "efT": efT,
            "elocT": np.ascontiguousarray(efT[:, aoff:aoff + NLOC]),
            "labrow": labr.reshape(1, N).astype(np.float32),
            "labT": labT,
            "llocrow": np.ascontiguousarray(
                labr[aoff:aoff + NLOC].reshape(1, NLOC)),
            "llocT": np.ascontiguousarray(
                labr[aoff:aoff + NLOC].reshape(NLOC, 1)),
            "eye": eye,
            "onesb": np.ones((128, NLOC), ml_dtypes.bfloat16),
        })
    return in_maps, nct_w


def run_on_device(embeddings: np.ndarray, labels: np.ndarray, **run_kwargs):
    from concourse.bass_utils import run_bass_kernel_spmd
    in_maps, nct_w = _make_inputs(embeddings, labels)
    nc = _get_program(nct_w)
    res = run_bass_kernel_spmd(nc, in_maps, core_ids=list(range(NCORES)),
                               **run_kwargs)
    rows_w = nct_w * 128        # j-rows per sign matrix
    total = 0.0
    count = 0.0
    for r in range(NCORES):
        o = res.results[r]["out"].astype(np.float64)
        tsum = res.results[r]["out2"].astype(np.float64).reshape(-1)
        dsum = o[0:NLOC, 4 * NGRP]
        for g in range(NGRP):
            na = min(GRPA, NLOC - GRPA * g)
            for m in range(na):
                a = GRPA * g + m
                qd, s8 = m // 8, m % 8
                bw = 32 * qd + 2 * s8
                bq = 32 * qd + 16 + s8
                w = q = p2 = 0.0
                for ch in range(2):
                    w += o[bw + 0, 2 * g + ch] + o[bw + 1, 2 * g + ch]
                    q += o[bq, 2 * g + ch]
                    p2 += o[bq, 2 * NGRP + 2 * g + ch]
                if _is_sign_anchor(a):
                    w = 0.5 * w + 0.5 * N * tsum[a]
                    q = 0.5 * q + 0.5 * rows_w * N
                    p2 = 0.5 * p2 + 0.5 * rows_w * dsum[a]
                total += w - p2
                count += q
    return total, count, res


def kernel(embeddings: np.ndarray, labels: np.ndarray):
    embeddings = np.asarray(embeddings)
    labels = np.asarray(labels)
    total, count, _ = run_on_device(embeddings, labels)

    lab = np.asarray(labels).reshape(-1)
    cnt = np.bincount(lab.astype(np.int64), minlength=1)
    per = cnt[lab.astype(np.int64)]
    num_valid = int(((per - 1) * (N - per)).sum())

    nv = np.float32(num_valid)
    ne = np.float32(count)
    tot = np.float32(total)
    if ne > 0:
        loss = np.float32(tot / np.maximum(ne, np.float32(1.0)))
    else:
        loss = np.float32(0.0)
    frac = np.float32(ne / (nv + np.float32(1e-16)))
    return (np.array(loss, np.float32), np.array(nv, np.float32),
            np.array(ne, np.float32), np.array(frac, np.float32))


# revision 25
# speedup vs baseline: 5.9686x; 5.9686x over previous
"""BatchAllTripletLoss on 8 Trainium2 NeuronCores via Bass/Tile.

Math: for anchors i, positives j (same label, j!=i), negatives k (diff label):
  total        = sum_{i,j,k} relu(d_ij - d_ik + margin)
  num_non_easy = #{(i,j,k): d_ik < d_ij + margin}
  loss         = total / num_non_easy ; frac = num_non_easy / num_valid

Sharding: samples are sorted by class; core r owns 80 consecutive anchors of
the sorted order. Each core additionally gets its OWN cyclic rotation of the
sorted sample order such that the classes its anchors belong to occupy
columns [0, W) with W <= 256 ("window"). All positives j of every local
anchor then live in the first NCT_W=2 column tiles, so the per-anchor
comparison matrix M[j, k] = (v'_k < t'_j) shrinks from [640, 640] to
[256, 640]. Loss sums/counts are permutation invariant, so each core can use
its own sample order.

Per anchor a:
  - masked row v'_k = d_ak + BIG*(same label), bf16, staged to DRAM and
    DMA-broadcast to [128, 8*640] for 8 anchors at a time.
  - masked thresholds t'_j = (d_aj + margin) * positive_mask over the window
    rows only, hi/lo split into bf16 lhsT tiles.
  - M built in one bf16 op per window tile: DVE tensor_scalar is_lt for 5 of
    8 anchors; ACT Sign (corrected on host) for 3 of 8.
  - PE reduces M with lhsT = [t'_hi | t'_lo | 1 | 0] (bf16), psum rows 4m..
    4m+3 for anchor m of a 32-anchor psum group (32 anchors share one bank
    pair; drains happen 3x per core instead of 27x).
  - drain: ACT free-sums psum rows; DVE fused (q * dist) reduce.
  total = sum(t'*M) - sum(d*q);  count = sum(q).  Host combines in f64.
num_valid is pure label counting (host, exact).
"""

import numpy as np
import ml_dtypes

N = 640
D = 128
NCORES = 8
NLOC = N // NCORES            # 80 anchors per core
GRPA = 24                     # anchors per psum group (3 quadrants x 8)
NGRP = (NLOC + GRPA - 1) // GRPA   # 4 groups: 24, 24, 24, 8
VBB = 8                       # anchors per v'-broadcast DMA
MARGIN = 1.9
BIG = 1.0e9


def _is_sign_anchor(a: int) -> bool:
    return a % 8 in (0, 3, 6)


_CACHE = {}


def _build_program(nct_w: int):
    import concourse.bass as bass
    import concourse.bacc as bacc
    import concourse.mybir as mybir
    import concourse.tile as tile
    from concourse.masks import make_identity

    f32 = mybir.dt.float32
    bf16 = mybir.dt.bfloat16
    Alu = mybir.AluOpType
    Act = mybir.ActivationFunctionType

    W = nct_w * 128               # window width (positive-candidate columns)

    nc = bacc.Bacc("TRN2", target_bir_lowering=False, debug=False,
                   num_devices=NCORES)

    efT = nc.declare_dram_parameter("efT", [D, N], f32, isOutput=False)
    elocT = nc.declare_dram_parameter("elocT", [D, NLOC], f32, isOutput=False)
    labrow = nc.declare_dram_parameter("labrow", [1, N], f32, isOutput=False)
    labT = nc.declare_dram_parameter("labT", [128, nct_w], f32, isOutput=False)
    llocrow = nc.declare_dram_parameter("llocrow", [1, NLOC], f32, isOutput=False)
    llocT = nc.declare_dram_parameter("llocT", [NLOC, 1], f32, isOutput=False)
    eye = nc.declare_dram_parameter("eye", [128, nct_w * NLOC], f32,
                                    isOutput=False)
    onesb = nc.declare_dram_parameter("onesb", [128, NLOC], bf16,
                                      isOutput=False)
    # out: [128, 2*NGRP(wsums) + 2*NGRP(p2) + 1 (dist row sums)]
    out_d = nc.declare_dram_parameter("out", [128, 4 * NGRP + 1], f32,
                                      isOutput=True)
    # out2: [1, NLOC] Tsum_a = sum_j t'_aj (for ACT-sign anchors' corrections)
    out2_d = nc.declare_dram_parameter("out2", [1, NLOC], f32, isOutput=True)

    from contextlib import ExitStack
    with tile.TileContext(nc) as tc:
        with (
            tc.tile_pool(name="singles", bufs=1) as sg,
            tc.tile_pool(name="vbp", bufs=3) as vbp,
            tc.tile_pool(name="mtp", bufs=10) as mtp,
            tc.tile_pool(name="dpp", bufs=2) as dpp,
            tc.tile_pool(name="drs", bufs=3) as drs,
            tc.tile_pool(name="dram", bufs=1, space="DRAM") as dram,
        ):
            pro_stack = ExitStack()
            ps_mm = pro_stack.enter_context(
                tc.tile_pool(name="ps_mm", bufs=1, space="PSUM"))
            ps_tr = pro_stack.enter_context(
                tc.tile_pool(name="ps_tr", bufs=1, space="PSUM"))
            # ---- load inputs ----
            EF = sg.tile([D, N], f32)
            nc.sync.dma_start(out=EF[:], in_=efT[:])
            EL = sg.tile([D, NLOC], f32)
            nc.sync.dma_start(out=EL[:], in_=elocT[:])
            LR = sg.tile([1, N], f32)
            nc.gpsimd.dma_start(out=LR[:], in_=labrow[:])
            LT = sg.tile([128, nct_w], f32)
            nc.gpsimd.dma_start(out=LT[:], in_=labT[:])
            LLR = sg.tile([1, NLOC], f32)
            nc.gpsimd.dma_start(out=LLR[:], in_=llocrow[:])
            LLT = sg.tile([NLOC, 1], f32)
            nc.gpsimd.dma_start(out=LLT[:], in_=llocT[:])
            EYE = sg.tile([128, nct_w * NLOC], f32)
            nc.sync.dma_start(out=EYE[:], in_=eye[:])
            # pre-allocate + zero the quadrant lhsT tiles early (no data dep)
            lhs32 = []
            row_d = 32 * NLOC
            for c in range(nct_w):
                L32 = sg.tile([128, 32, NLOC], bf16, tag=f"l32_{c}",
                              name=f"l32_{c}")
                nc.gpsimd.memset(L32[:], 0.0)
                for s in range(8):
                    dst_o = bass.AP(tensor=L32[:].tensor,
                                    offset=L32[:].offset + (16 + s) * NLOC + s,
                                    ap=[[row_d, 128], [8, NLOC // 8]])
                    nc.gpsimd.memset(dst_o, 1.0)
                lhs32.append(L32)

            ident = sg.tile([128, 128], f32)
            make_identity(nc, ident[:])
            ones = sg.tile([128, 1], f32)
            nc.vector.memset(ones[:], 1.0)

            # ---- pairwise distance rows for local anchors ----
            Esq = sg.tile([D, N], f32)
            nc.vector.tensor_mul(Esq[:], EF[:], EF[:])
            ELsq = sg.tile([D, NLOC], f32)
            nc.vector.tensor_mul(ELsq[:], EL[:], EL[:])

            sqf_ps = ps_mm.tile([1, N], f32, tag="pro", name="sqf")
            nc.tensor.matmul(sqf_ps[:, 0:512], ones[:], Esq[:, 0:512])
            nc.tensor.matmul(sqf_ps[:, 512:N], ones[:], Esq[:, 512:N])
            SQF = sg.tile([1, N], f32)
            nc.vector.tensor_copy(SQF[:], sqf_ps[:])

            sql_ps = ps_mm.tile([NLOC, 1], f32, tag="pro", name="sql")
            nc.tensor.matmul(sql_ps[:], ELsq[:], ones[:])
            SQL = sg.tile([NLOC, 1], f32)
            nc.vector.tensor_copy(SQL[:], sql_ps[:])

            dot_ps = ps_mm.tile([NLOC, N], f32, tag="pro", name="dot")
            nc.tensor.matmul(dot_ps[:, 0:512], EL[:], EF[:, 0:512])
            nc.tensor.matmul(dot_ps[:, 512:N], EL[:], EF[:, 512:N])

            A = sg.tile([NLOC, N], f32)
            nc.vector.tensor_scalar(out=A[:], in0=dot_ps[:], scalar1=-2.0,
                                    scalar2=SQL[:], op0=Alu.mult, op1=Alu.add)
            sqf_d = dram.tile([1, N], f32)
            nc.sync.dma_start(out=sqf_d[:], in_=SQF[:])
            SQB = sg.tile([128, N], f32)
            nc.sync.dma_start(out=SQB[0:NLOC, :],
                              in_=sqf_d[:].to_broadcast([NLOC, N]))
            PRE = sg.tile([NLOC, N], f32)
            nc.vector.tensor_add(PRE[:], A[:], SQB[0:NLOC, :])
            nc.vector.tensor_scalar(out=PRE[:], in0=PRE[:], scalar1=0.0,
                                    scalar2=None, op0=Alu.max)
            DIST = sg.tile([NLOC, N], f32)
            nc.scalar.activation(out=DIST[:], in_=PRE[:], func=Act.Sqrt)

            # masked v' row, bf16
            LBC = sg.tile([128, N], f32)
            nc.sync.dma_start(out=LBC[0:NLOC, :],
                              in_=labrow[:].to_broadcast([NLOC, N]))
            EQB = sg.tile([NLOC, N], f32)
            nc.vector.tensor_scalar(out=EQB[:], in0=LBC[0:NLOC, :], scalar1=LLT[:],
                                    scalar2=BIG, op0=Alu.is_equal, op1=Alu.mult)
            VM = sg.tile([NLOC, N], f32)
            nc.vector.tensor_add(VM[:], DIST[:], EQB[:])
            VMB = sg.tile([NLOC, N], bf16)
            nc.vector.tensor_copy(VMB[:], VM[:])
            vmd = dram.tile([NLOC, N], bf16)
            nc.sync.dma_start(out=vmd[:], in_=VMB[:])

            # positive mask transposed: (lab_j == lab_a) - eye  (window only)
            LLB = sg.tile([128, NLOC], f32)
            nc.sync.dma_start(out=LLB[:],
                              in_=llocrow[:].to_broadcast([128, NLOC]))
            posT = []
            for c in range(nct_w):
                p = sg.tile([128, NLOC], f32, tag=f"posT{c}", name=f"posT{c}")
                nc.vector.tensor_scalar(out=p[:], in0=LLB[:], scalar1=LT[:, c:c + 1],
                                        scalar2=None, op0=Alu.is_equal)
                nc.vector.tensor_sub(p[:], p[:], EYE[:, c * NLOC:(c + 1) * NLOC])
                posT.append(p)

            # thresholds: tp[c][p, a] = (dist[a, c*128+p] + margin) * posT
            # plus bf16 hi/lo split packed into lhsT tiles [128, NLOC, 4],
            # then scattered via DRAM into zero-padded [128, NLOC, 32] tiles
            # so 8 anchors can share one PE output quadrant (anchor a's 4
            # cols live at 4*(a%8); the rest are zeros).
            tp = []
            for c in range(nct_w):
                tr_ps = ps_tr.tile([128, NLOC], f32, tag="tr")
                nc.tensor.transpose(tr_ps[:], DIST[:, c * 128:(c + 1) * 128],
                                    ident[0:NLOC, 0:NLOC])
                t = sg.tile([128, NLOC], f32, tag=f"tp{c}", name=f"tp{c}")
                nc.vector.tensor_scalar_add(out=t[:], in0=tr_ps[:], scalar1=MARGIN)
                nc.vector.tensor_mul(t[:], t[:], posT[c][:])
                tp.append(t)

                TH = sg.tile([128, NLOC], bf16, tag="thb", name="thb")
                nc.vector.tensor_copy(TH[:], t[:])                 # t_hi (bf16)
                thf = sg.tile([128, NLOC], f32, tag="thf", name="thf")
                nc.vector.tensor_copy(thf[:], TH[:])               # back to f32
                nc.vector.tensor_sub(thf[:], t[:], thf[:])         # t_lo
                TL = sg.tile([128, NLOC], bf16, tag="tlb", name="tlb")
                nc.vector.tensor_copy(TL[:], thf[:])
                # fill zero-padded quadrant lhsT (layout [128, col, a]):
                # anchor slot s=a%8 has (hi, lo) at cols (2s, 2s+1); ones
                # pre-filled at col 16+s. matmul reads col-strided.
                L32 = lhs32[c]
                row_s = NLOC
                for s in range(8):
                    nhere = NLOC // 8
                    src_h = bass.AP(tensor=TH[:].tensor, offset=TH[:].offset + s,
                                    ap=[[row_s, 128], [8, nhere]])
                    dst_h = bass.AP(tensor=L32[:].tensor,
                                    offset=L32[:].offset + 2 * s * NLOC + s,
                                    ap=[[row_d, 128], [8, nhere]])
                    nc.vector.tensor_copy(dst_h, src_h)
                    src_l = bass.AP(tensor=TL[:].tensor, offset=TL[:].offset + s,
                                    ap=[[row_s, 128], [8, nhere]])
                    dst_l = bass.AP(tensor=L32[:].tensor,
                                    offset=L32[:].offset + (2 * s + 1) * NLOC + s,
                                    ap=[[row_d, 128], [8, nhere]])
                    nc.gpsimd.tensor_copy(dst_l, src_l)

            # dist row sums (for sign-anchor corrections) -> OUTS directly
            OUTS = sg.tile([128, 4 * NGRP + 1], f32)
            DSC = sg.tile([NLOC, N], f32)
            nc.scalar.activation(out=DSC[:], in_=DIST[:], func=Act.Identity,
                                 bias=0.0, scale=1.0,
                                 accum_out=OUTS[0:NLOC, 4 * NGRP:4 * NGRP + 1])

            # Tsum_a = sum_j t'_aj : ones^T @ tp[c], accumulated over c
            ts_ps = ps_tr.tile([1, NLOC], f32, tag="tr", name="ts_ps")
            for c in range(nct_w):
                nc.tensor.matmul(ts_ps[:], ones[:], tp[c][:],
                                 start=(c == 0), stop=(c == nct_w - 1))
            TSROW = sg.tile([1, NLOC], f32)
            nc.vector.tensor_copy(TSROW[:], ts_ps[:])
            nc.sync.dma_start(out=out2_d[:], in_=TSROW[:])

            pro_stack.close()
            wq_stack = ExitStack()
            ps_wq1 = wq_stack.enter_context(
                tc.tile_pool(name="ps_wq1", bufs=2, space="PSUM"))
            ps_wq2 = wq_stack.enter_context(
                tc.tile_pool(name="ps_wq2", bufs=2, space="PSUM"))

            # ---- main loop ----
            vb_cache = {}
            for g in range(NGRP):
                na = min(GRPA, NLOC - GRPA * g)
                nqd = (na + 7) // 8
                wq1 = ps_wq1.tile([128, 512], f32, tag="wq1", name="wq1")
                wq2 = ps_wq2.tile([128, 128], f32, tag="wq2", name="wq2")
                dp = dpp.tile([128, N], f32, tag="dp", name="dp")
                # dist rows of quadrant's anchors -> dp partitions 32*qd+16+s
                for qd in range(nqd):
                    bn = min(8, na - 8 * qd)
                    a0 = GRPA * g + 8 * qd
                    nc.sync.dma_start(
                        out=dp[32 * qd + 16:32 * qd + 16 + bn, :],
                        in_=DIST[a0:a0 + bn, :])
                for m in range(na):
                    a = GRPA * g + m
                    qd, s8 = m // 8, m % 8
                    bn = min(8, na - 8 * qd)
                    if a % VBB == 0:
                        vb2 = vbp.tile([128, VBB, N], bf16, tag="vb", name="vb")
                        sl = vmd[a:a + VBB, :]
                        bsrc = bass.AP(tensor=sl.tensor, offset=sl.offset,
                                       ap=[[0, 128]] + [list(q) for q in sl.ap])
                        nc.sync.dma_start(out=vb2[:], in_=bsrc)
                        vb_cache[0] = vb2
                    vb = vb_cache[0][:, a % VBB, :]
                    on_act = _is_sign_anchor(a)
                    first = (s8 == 0)
                    last = (s8 == bn - 1)
                    for c in range(nct_w):
                        mt = mtp.tile([128, N], bf16, tag="mt", name="mt")
                        if on_act:
                            nc.scalar.activation(out=mt[:], in_=vb[:],
                                                 func=Act.Sign,
                                                 bias=tp[c][:, a:a + 1],
                                                 scale=-1.0)
                        else:
                            nc.vector.tensor_scalar(out=mt[:], in0=vb[:],
                                                    scalar1=tp[c][:, a:a + 1],
                                                    scalar2=None, op0=Alu.is_lt)
                        st = first and (c == 0)
                        sp = last and (c == nct_w - 1)
                        nc.tensor.matmul(wq1[32 * qd:32 * qd + 32, :],
                                         lhs32[c][:, :, a], mt[:, 0:512],
                                         start=st, stop=sp)
                        nc.tensor.matmul(wq2[32 * qd:32 * qd + 32, :],
                                         lhs32[c][:, :, a], mt[:, 512:N],
                                         start=st, stop=sp)
                # drain group: ACT free-sums all psum rows; DVE fused q*dist
                sa1 = drs.tile([128, 512], f32, tag="sa1", name="sa1")
                sa2 = drs.tile([128, 128], f32, tag="sa2", name="sa2")
                sb1 = drs.tile([128, 512], f32, tag="sb1", name="sb1")
                sb2 = drs.tile([128, 128], f32, tag="sb2", name="sb2")
                nc.scalar.activation(out=sa1[:], in_=wq1[:], func=Act.Identity,
                                     bias=0.0, scale=1.0,
                                     accum_out=OUTS[:, 2 * g:2 * g + 1])
                nc.scalar.activation(out=sa2[:], in_=wq2[:], func=Act.Identity,
                                     bias=0.0, scale=1.0,
                                     accum_out=OUTS[:, 2 * g + 1:2 * g + 2])
                nc.vector.scalar_tensor_tensor(out=sb1[:], in0=wq1[:],
                                               scalar=1.0, in1=dp[:, 0:512],
                                               op0=Alu.mult, op1=Alu.mult,
                                               accum_out=OUTS[:, 2 * NGRP + 2 * g:
                                                              2 * NGRP + 2 * g + 1])
                nc.vector.scalar_tensor_tensor(out=sb2[:], in0=wq2[:],
                                               scalar=1.0, in1=dp[:, 512:N],
                                               op0=Alu.mult, op1=Alu.mult,
                                               accum_out=OUTS[:, 2 * NGRP + 2 * g + 1:
                                                              2 * NGRP + 2 * g + 2])

            nc.gpsimd.dma_start(out=out_d[:], in_=OUTS[:])
            wq_stack.close()

    nc.compile()
    return nc


def _get_program(nct_w: int):
    key = ("nc", nct_w)
    if key not in _CACHE:
        _CACHE[key] = _build_program(nct_w)
    return _CACHE[key]


def _plan_layout(lab: np.ndarray):
    """Class-sort the samples; per core build a rotated order that puts the
    window (classes of its anchors) at columns [0, W)."""
    order = np.argsort(lab, kind="stable")
    slab = lab[order]
    perms = []
    aoffs = []
    maxw = 0
    for r in range(NCORES):
        lo = slab[NLOC * r]
        hi = slab[NLOC * r + NLOC - 1]
        w0 = int(np.searchsorted(slab, lo, "left"))
        w1 = int(np.searchsorted(slab, hi, "right"))
        maxw = max(maxw, w1 - w0)
        perm = np.concatenate([order[w0:], order[:w0]])
        perms.append(perm)
        aoffs.append(NLOC * r - w0)
    nct_w = max(1, (maxw + 127) // 128)
    return perms, aoffs, nct_w


def _make_inputs(embeddings: np.ndarray, labels: np.ndarray):
    e = np.ascontiguousarray(embeddings.reshape(N, D).astype(np.float32))
    lab = labels.reshape(N).astype(np.float32)
    perms, aoffs, nct_w = _plan_layout(lab)

    in_maps = []
    for r in range(NCORES):
        perm = perms[r]
        aoff = aoffs[r]
        er = e[perm]                                     # [N, D] core order
        efT = np.ascontiguousarray(er.T)                 # [D, N]
        labr = lab[perm]
        eye = np.zeros((128, nct_w * NLOC), np.float32)
        for a in range(NLOC):
            j = aoff + a
            eye[j % 128, (j // 128) * NLOC + a] = 1.0
        labT = np.zeros((128, nct_w), np.float32)
        labT[:, :] = labr[:nct_w * 128].reshape(nct_w, 128).T
        in_maps.append({
            "efT": efT,
            "elocT": np.ascontiguousarray(efT[:, aoff:aoff + NLOC]),
            "labrow": labr.reshape(1, N).astype(np.float32),
            "labT": labT,
            "llocrow": np.ascontiguousarray(
                labr[aoff:aoff + NLOC].reshape(1, NLOC)),
            "llocT": np.ascontiguousarray(
                labr[aoff:aoff + NLOC].reshape(NLOC, 1)),
            "eye": eye,
            "onesb": np.ones((128, NLOC), ml_dtypes.bfloat16),
        })
    return in_maps, nct_w


def run_on_device(embeddings: np.ndarray, labels: np.ndarray, **run_kwargs):
    from concourse.bass_utils import run_bass_kernel_spmd
    in_maps, nct_w = _make_inputs(embeddings, labels)
    nc = _get_program(nct_w)
    res = run_bass_kernel_spmd(nc, in_maps, core_ids=list(range(NCORES)),
                               **run_kwargs)
    rows_w = nct_w * 128        # j-rows per sign matrix
    total = 0.0
    count = 0.0
    for r in range(NCORES):
        o = res.results[r]["out"].astype(np.float64)
        tsum = res.results[r]["out2"].astype(np.float64).reshape(-1)
        dsum = o[0:NLOC, 4 * NGRP]
        for g in range(NGRP):
            na = min(GRPA, NLOC - GRPA * g)
            for m in range(na):
                a = GRPA * g + m
                qd, s8 = m // 8, m % 8
                bw = 32 * qd + 2 * s8
                bq = 32 * qd + 16 + s8
                w = q = p2 = 0.0
                for ch in range(2):
                    w += o[bw + 0, 2 * g + ch] + o[bw + 1, 2 * g + ch]
                    q += o[bq, 2 * g + ch]
                    p2 += o[bq, 2 * NGRP + 2 * g + ch]
                if _is_sign_anchor(a):
                    w = 0.5 * w + 0.5 * N * tsum[a]
                    q = 0.5 * q + 0.5 * rows_w * N
                    p2 = 0.5 * p2 + 0.5 * rows_w * dsum[a]
                total += w - p2
                count += q
    return total, count, res


def kernel(embeddings: np.ndarray, labels: np.ndarray):
    embeddings = np.asarray(embeddings)
    labels = np.asarray(labels)
    total, count, _ = run_on_device(embeddings, labels)

    lab = np.asarray(labels).reshape(-1)
    cnt = np.bincount(lab.astype(np.int64), minlength=1)
    per = cnt[lab.astype(np.int64)]
    num_valid = int(((per - 1) * (N - per)).sum())

    nv = np.float32(num_valid)
    ne = np.float32(count)
    tot = np.float32(total)
    if ne > 0:
        loss = np.float32(tot / np.maximum(ne, np.float32(1.0)))
    else:
        loss = np.float32(0.0)
    frac = np.float32(ne / (nv + np.float32(1e-16)))
    return (np.array(loss, np.float32), np.array(nv, np.float32),
            np.array(ne, np.float32), np.array(frac, np.float32))
